# revision 1
# baseline (speedup 1.0000x reference)
"""Trainium2 Bass kernel for nn_Decoder_33208687133135.

Reference computation (B=2048, D=64, L=64, H=512):
    z = swapaxes(koopman, 1, 2)                    # (B, D, L)
    s = MLP_s(z); t = MLP_t(z)                     # (B, D, D), 4 layers, tanh
    ds = diag(s); dt = diag(t)                     # (B, D)
    out = (x - dt) * exp(-ds)

Key structural insight: only the diagonal of the (B, D, D) MLP outputs is
needed, so layer 4 reduces to a per-row dot product with a single W4 column.

Layout: "feature-major" (transposed) activations.  Rows of the fused
(B*D)-row MLP are processed in blocks of 512 with a FIXED latent index i per
block; activations live as [features(partition), rows(free)] tiles so no
transposes are ever needed between layers.  Layer 4 then needs only W4[:, i]
per block: 4 accumulating K=128 matvecs -> psum [1, 512] = ds (or dt) for the
whole block.  exp/sub/mul run on ACT/DVE straight out of PSUM.

Sharding: latent-parallel.  Core m handles i in [8m, 8m+8) for all 2048
batches = 16384 rows = 32 blocks.  MLP weights are replicated (cast bf16).

All matmuls are bf16 (fp32 PSUM accumulation); measured l2 rel err vs the
fp32 reference is ~2.7e-3.  Measured HW exec time: ~593 us per core (8 cores
in parallel), ~93% TensorE occupancy at the N=512-column streaming rate.
"""

import numpy as np
import ml_dtypes

import concourse.mybir as mybir
import concourse.tile as tile
from concourse import bacc
from concourse.bass_utils import run_bass_kernel_spmd

BF16 = mybir.dt.bfloat16
F32 = mybir.dt.float32
_bf = ml_dtypes.bfloat16

B, D, L, H = 2048, 64, 64, 512
NCORES = 8
IPC = D // NCORES          # latent indices per core (8)
BN = 512                   # rows (batches) per block
BPI = B // BN              # blocks per latent index (4)
NBLK = IPC * BPI           # blocks per core (32)
NROW = IPC * B             # rows per core (16384)

_CACHE = {}


def _build_nc():
    """Build the (single) SPMD Bass program; identical on all 8 cores."""
    nc = bacc.Bacc("TRN2", target_bir_lowering=False, debug=False,
                   num_devices=NCORES)

    Tanh = mybir.ActivationFunctionType.Tanh
    Exp = mybir.ActivationFunctionType.Exp

    z2_d = nc.dram_tensor("z2", [L, NROW], BF16, kind="ExternalInput").ap()
    w1_d = nc.dram_tensor("w1", [2, L, H], BF16, kind="ExternalInput").ap()
    w2_d = nc.dram_tensor("w2", [2, H, H], BF16, kind="ExternalInput").ap()
    w3_d = nc.dram_tensor("w3", [2, H, H], BF16, kind="ExternalInput").ap()
    l4_d = nc.dram_tensor("l4", [2, H, NBLK], BF16, kind="ExternalInput").ap()
    b123_d = nc.dram_tensor("b123", [2, 3, 128, 4], F32, kind="ExternalInput").ap()
    eb_d = nc.dram_tensor("eb", [1, NBLK], F32, kind="ExternalInput").ap()
    xa_d = nc.dram_tensor("xa", [1, NROW], F32, kind="ExternalInput").ap()
    out_d = nc.dram_tensor("out", [NBLK, BN], F32, kind="ExternalOutput").ap()

    with tile.TileContext(nc) as tc:
        with (
            tc.tile_pool(name="const", bufs=1) as const,
            tc.tile_pool(name="hpool", bufs=12) as hpool,
            tc.tile_pool(name="fin", bufs=4) as fin,
            tc.tile_pool(name="psum", bufs=7, space="PSUM") as psum,
            tc.tile_pool(name="psd", bufs=1, space="PSUM") as psd,
        ):
            # --- constants; DMA order matters: first block's needs first,
            # split across HWDGE queues (each queue runs ~14 GB/s) ---
            w1_t = [const.tile([L, H], BF16, tag=f"w1_{mi}", name=f"w1_{mi}")
                    for mi in range(2)]
            b_t = [[const.tile([128, 4], F32, tag=f"b_{mi}_{ly}", name=f"b_{mi}_{ly}")
                    for ly in range(3)] for mi in range(2)]
            zbig = const.tile([L, NROW], BF16, tag="z")
            w2_t = [[const.tile([128, H], BF16, tag=f"w2_{mi}_{kc}", name=f"w2_{mi}_{kc}")
                     for kc in range(4)] for mi in range(2)]
            w3_t = [[const.tile([128, H], BF16, tag=f"w3_{mi}_{kc}", name=f"w3_{mi}_{kc}")
                     for kc in range(4)] for mi in range(2)]
            l4_t = [[const.tile([128, NBLK], BF16, tag=f"l4_{mi}_{kc}", name=f"l4_{mi}_{kc}")
                     for kc in range(4)] for mi in range(2)]
            eb_t = const.tile([1, NBLK], F32, tag="eb")
            xa_t = const.tile([1, NROW], F32, tag="xa")

            def zdma(c0, c1):
                nc.sync.dma_start(zbig[:, c0:c1], z2_d[:, c0:c1])

            nc.sync.dma_start(zbig[0:32, 0:BN], z2_d[0:32, 0:BN])
            nc.sync.dma_start(zbig[32:64, 0:BN], z2_d[32:64, 0:BN])
            nc.sync.dma_start(w1_t[0][0:32, :], w1_d[0][0:32, :])
            nc.sync.dma_start(w1_t[0][32:64, :], w1_d[0][32:64, :])
            for ly in range(3):
                nc.sync.dma_start(b_t[0][ly][:], b123_d[0, ly])
            for kc in range(4):
                nc.sync.dma_start(w2_t[0][kc][:],
                                  w2_d[0, kc * 128:(kc + 1) * 128, :])
            nc.sync.dma_start(w1_t[1][:], w1_d[1])
            for ly in range(3):
                nc.sync.dma_start(b_t[1][ly][:], b123_d[1, ly])
            for kc in range(4):
                nc.sync.dma_start(w2_t[1][kc][:],
                                  w2_d[1, kc * 128:(kc + 1) * 128, :])
            for mi in range(2):
                for kc in range(4):
                    nc.sync.dma_start(w3_t[mi][kc][:],
                                      w3_d[mi, kc * 128:(kc + 1) * 128, :])
            for mi in range(2):
                for kc in range(4):
                    nc.sync.dma_start(l4_t[mi][kc][:],
                                      l4_d[mi, kc * 128:(kc + 1) * 128, :])
            for c in range(BN, 4 * BN, BN):  # blocks 1-3
                zdma(c, c + BN)
            nc.sync.dma_start(eb_t[:], eb_d)
            nc.sync.dma_start(xa_t[:], xa_d)
            for s in range(1, 8):            # blocks 4-31
                zdma(s * (NROW // 8), (s + 1) * (NROW // 8))

            for j in range(NBLK):
                zv = zbig[:, j * BN:(j + 1) * BN]
                h_prev = [None, None]
                pd_sd = []
                # layer 1 (s then t), interleaved by layer for smoother ACT flow
                for mi in range(2):
                    h_prev[mi] = []
                    for f in range(4):
                        p = psum.tile([128, BN], F32, tag="mm")
                        nc.tensor.matmul(p[:], w1_t[mi][:, f * 128:(f + 1) * 128],
                                         zv, start=True, stop=True)
                        h = hpool.tile([128, BN], BF16, tag=f"h{f}", name=f"h_{j}_{mi}_0_{f}")
                        nc.scalar.activation(h[:], p[:], Tanh,
                                             bias=b_t[mi][0][:, f:f + 1])
                        h_prev[mi].append(h)
                for ly, w_t2 in ((1, w2_t), (2, w3_t)):
                    for mi in range(2):
                        # f-pairs with kc-major inside the pair: MM (f, kc)
                        # needs only h_prev[kc], so the first MMs of the layer
                        # don't wait for the last tanh of the previous layer,
                        # while only 2 psum tiles stay live at a time
                        h_next = []
                        for fp in range(2):
                            ps = [psum.tile([128, BN], F32, tag="mm",
                                            name=f"p_{j}_{mi}_{ly}_{fp}_{c}")
                                  for c in range(2)]
                            for kc in range(4):
                                for c in range(2):
                                    f = 2 * fp + c
                                    nc.tensor.matmul(
                                        ps[c][:],
                                        w_t2[mi][kc][:, f * 128:(f + 1) * 128],
                                        h_prev[mi][kc][:],
                                        start=(kc == 0), stop=(kc == 3))
                            for c in range(2):
                                f = 2 * fp + c
                                h = hpool.tile([128, BN], BF16, tag=f"h{f}", name=f"h_{j}_{mi}_{ly}_{f}")
                                nc.scalar.activation(h[:], ps[c][:], Tanh,
                                                     bias=b_t[mi][ly][:, f:f + 1])
                                h_next.append(h)
                        h_prev[mi] = h_next
                for mi in range(2):
                    pd = psd.tile([1, BN], F32, tag="pd")
                    for kc in range(4):
                        nc.tensor.matmul(pd[:], l4_t[mi][kc][:, j:j + 1],
                                         h_prev[mi][kc][:],
                                         start=(kc == 0), stop=(kc == 3))
                    pd_sd.append(pd)
                # out_block = (x - dt) * exp(-ds)   [all biases folded in]
                es = fin.tile([1, BN], F32, tag="es")
                nc.scalar.activation(es[:], pd_sd[0][:], Exp, scale=-1.0,
                                     bias=eb_t[:, j:j + 1])
                tmp = fin.tile([1, BN], F32, tag="tmp")
                nc.vector.tensor_sub(tmp[:], xa_t[:, j * BN:(j + 1) * BN],
                                     pd_sd[1][:])
                outt = fin.tile([1, BN], F32, tag="outt")
                nc.vector.tensor_mul(outt[:], tmp[:], es[:])
                nc.sync.dma_start(out_d[j:j + 1, :], outt[:])

    nc.compile()
    return nc


def _prep_in_maps(inputs):
    """Host-side sharding: slice/cast per-core input arrays."""
    f32 = np.float32
    g = {k: np.asarray(v, f32) for k, v in inputs.items()}
    koopman, x = g["koopman"], g["x"]

    # z2[l, i, b] = koopman[b, l, i]; bf16 once, then slice per core
    kt = np.ascontiguousarray(koopman.transpose(1, 2, 0)).astype(_bf)
    xT = np.ascontiguousarray(x.T)  # [D, B]

    w1 = np.stack([g["sW1"], g["tW1"]]).astype(_bf)
    w2 = np.stack([g["sW2"], g["tW2"]]).astype(_bf)
    w3 = np.stack([g["sW3"], g["tW3"]]).astype(_bf)
    w4 = np.stack([g["sW4"], g["tW4"]])  # keep f32; cast after column select
    b123 = np.empty((2, 3, 128, 4), f32)
    for mi, p in enumerate("st"):
        for ly in range(3):
            b123[mi, ly] = g[f"{p}b{ly + 1}"].reshape(4, 128).T
    b4s, b4t = g["sb4"], g["tb4"]

    in_maps = []
    for m in range(NCORES):
        i0 = m * IPC
        z2c = np.ascontiguousarray(kt[:, i0:i0 + IPC, :]).reshape(L, NROW)
        l4 = np.repeat(w4[:, :, i0:i0 + IPC], BPI, axis=2).astype(_bf)
        eb = np.repeat(-b4s[i0:i0 + IPC], BPI).astype(f32).reshape(1, NBLK)
        xa = (xT[i0:i0 + IPC] - b4t[i0:i0 + IPC, None]).astype(f32)
        in_maps.append({
            "z2": z2c,
            "w1": w1, "w2": w2, "w3": w3, "l4": l4,
            "b123": b123, "eb": eb,
            "xa": np.ascontiguousarray(xa).reshape(1, NROW),
        })
    return in_maps


def _run(inputs, **run_kwargs):
    if "nc" not in _CACHE:
        _CACHE["nc"] = _build_nc()
    nc = _CACHE["nc"]
    in_maps = _prep_in_maps(inputs)
    res = run_bass_kernel_spmd(nc, in_maps, core_ids=list(range(NCORES)),
                               **run_kwargs)
    outT = np.empty((D, B), np.float32)
    for m in range(NCORES):
        i0 = m * IPC
        outT[i0:i0 + IPC] = np.asarray(
            res.results[m]["out"], np.float32).reshape(IPC, B)
    return np.ascontiguousarray(outT.T), res


def kernel(**inputs) -> np.ndarray:
    out, _ = _run(inputs)
    return out

